# revision 20
# baseline (speedup 1.0000x reference)
"""Trainium2 Bass kernel for nn_FCBlock (dense MLP, 8-core data parallel).

Full (unsharded) inputs in, full output out. Internally: coords are sharded
along the batch axis across 8 NeuronCores, the (few-MB) weights are
replicated, and each core runs a fused 4-stage MLP entirely out of SBUF.

Math (per point, D=3, H=512, L=3):
  y = coords[:, 3:], u = coords[:, :3]
  z = relu(10p*(u@Wzu0 + bzu0 + (y*(u@Wyuu0+byuu0))@Wzyu[0]))
  u = relu(10pu*(u@Wu0 + bu0))
  for i in 1..3:
    zz = z * relu(10pzu*(u@Wzuu[i-1] + bzuu[i-1]))
    yy = y * (u@Wyuu[i-1] + byuu[i-1])
    t  = zz@Wzzu[i-1] + u@Wzu[i-1] + bzu[i-1] + yy@Wzyu[i]     (i<3)
    t  = zz@Wzzu_last + u@Wzu_last + bzu_last + yy@Wzyu_last   (i=3 -> output)
    z, u = relu(10p*t), relu(10pu*(u@Wu[i-1] + bu[i-1]))

Layer 0 is folded on the host into a single K=8 matmul: the moving tensor
carries rows [u, 1, y*u, y], so u@Wzu0 + bzu0 + (y*(u@Wyuu0+byuu0))@Wzyu[0]
and u@Wu0 + bu0 are each one matmul with all biases and the rank-1
Wyuu0@Wzyu0 outer product folded into the stationary.  The final
yy@Wzyu_last (1x1 weight) is folded into a DVE op.  All 10p/10pu/10pzu
scales are folded into weights/biases on the host.

Device layout: activations are [H (4x128 partitions), B_tile (free)], weights
stationary [K<=128, M<=128]. Matmuls emit in 512-wide moving slices (PSUM
bank limit); ACT/DVE epilogues run over the full BT-wide tile.
"""

import sys

sys.path.insert(0, "/opt/trn_rl_repo")

import numpy as np

import concourse.bacc as bacc
import concourse.mybir as mybir
from concourse.tile import TileContext
from concourse.bass_utils import run_bass_kernel_spmd

NCORES = 8
NFULL = 65536
NC_B = NFULL // NCORES  # 8192 points per core
H = 512
L = 3
D = 3
P = 128
HO = H // P  # 4 partition-tiles per hidden vector
BT = 1024  # batch tile (ACT/DVE op width)
MW = 512   # matmul moving-slice width (PSUM bank limit)
NH = BT // MW
NT = NC_B // BT

F16 = mybir.dt.float16
F32 = mybir.dt.float32
RELU = mybir.ActivationFunctionType.Relu
ADD = mybir.AluOpType.add
MULT = mybir.AluOpType.mult

_CACHE = {}


def _build():
    nc = bacc.Bacc(trn_type="TRN2", name="fcblock2")

    m_d = nc.dram_tensor("m", [8, NC_B], F16, kind="ExternalInput")
    wbig = nc.dram_tensor("wbig", [9, H, H], F16, kind="ExternalInput")
    w0_d = nc.dram_tensor("w0", [8, 2 * H], F16, kind="ExternalInput")
    wy_d = nc.dram_tensor("wy", [H, L], F32, kind="ExternalInput")
    wzyv_d = nc.dram_tensor("wzyv", [2, H], F32, kind="ExternalInput")
    wl3_d = nc.dram_tensor("wl3", [H, 2], F32, kind="ExternalInput")
    bzuu_d = nc.dram_tensor("bzuu", [L, H], F32, kind="ExternalInput")
    bu_d = nc.dram_tensor("bu", [2, H], F32, kind="ExternalInput")
    bzu_d = nc.dram_tensor("bzu", [2, H], F32, kind="ExternalInput")
    byuu_d = nc.dram_tensor("byuu", [1, 4], F32, kind="ExternalInput")
    out = nc.dram_tensor("out", [NC_B, 1], F32, kind="ExternalOutput")

    with TileContext(nc) as tc:
        with (
            tc.tile_pool(name="wpool", bufs=1) as wpool,
            tc.tile_pool(name="spool", bufs=4) as spool,
            tc.tile_pool(name="psum", bufs=4, space="PSUM") as psum,
        ):
            # --- resident weights/biases ------------------------------------
            # Tile-0/1 inputs and w0 first: layer 0 of tile 0 needs only
            # these (tiny) DMAs, so they must not queue behind the rest.
            pre_io = {}
            for t in (0, 1):
                mt = spool.tile([8, BT], F16, name="mt", tag="mt", bufs=3)
                nc.sync.dma_start(mt[:], m_d[:, t * BT:(t + 1) * BT])
                pre_io[t] = mt
            w0 = wpool.tile([8, 2 * H], F16, name="w0")
            nc.sync.dma_start(w0[:], w0_d[:, :])
            wy = wpool.tile([P, HO, L], F32, name="wy")
            nc.sync.dma_start(wy[:], wy_d.rearrange("(ko p) s -> p ko s", p=P))
            wzyv = wpool.tile([P, 2, HO], F32, name="wzyv")
            nc.sync.dma_start(wzyv[:], wzyv_d.rearrange("s (ho p) -> p s ho", p=P))
            wl3 = wpool.tile([P, HO, 2], F32, name="wl3")
            nc.sync.dma_start(wl3[:], wl3_d.rearrange("(ko p) s -> p ko s", p=P))
            bzuu_t = wpool.tile([P, L, HO], F32, name="bzuu_t")
            nc.sync.dma_start(bzuu_t[:], bzuu_d.rearrange("s (ho p) -> p s ho", p=P))
            bu_t = wpool.tile([P, 2, HO], F32, name="bu_t")
            nc.sync.dma_start(bu_t[:], bu_d.rearrange("s (ho p) -> p s ho", p=P))
            bzu_t = wpool.tile([P, 2, HO], F32, name="bzu_t")
            nc.sync.dma_start(bzu_t[:], bzu_d.rearrange("s (ho p) -> p s ho", p=P))
            byuu_t = wpool.tile([1, 4], F32, name="byuu_t")
            nc.sync.dma_start(byuu_t[:], byuu_d[:, :])
            ones128 = wpool.tile([P, 1], F16, name="ones128")
            nc.gpsimd.memset(ones128[:], 1.0)
            # Big dense weights, one DMA per layer-slice so the layer-1
            # consumers arrive before the tail ones.
            wb = wpool.tile([P, 9, HO, H], F16, name="wb")
            for s in (0, 3, 5, 7, 1, 4, 6, 8, 2):
                nc.sync.dma_start(
                    wb[:, s], wbig[s].rearrange("(ko p) m -> p ko m", p=P))

            def mac_chain(terms):
                # per-partition MAC chain on DVE: acc = sum_i tens_i*scal_i;
                # a single ones-column matmul then reduces across partitions.
                a = spool.tile([P, BT], F16, name="pya", tag="pya", bufs=2)
                t0, s0 = terms[0]
                nc.vector.tensor_scalar_mul(a[:], t0, s0)
                for tn, sn in terms[1:]:
                    b = spool.tile([P, BT], F16, name="pyb", tag="pyb", bufs=2)
                    nc.vector.scalar_tensor_tensor(
                        b[:], tn, sn, a[:], op0=MULT, op1=ADD)
                    a = b
                return a

            def py_dve(u, col):
                return mac_chain(
                    [(u[:, ho, :], wy[:, ho, col:col + 1]) for ho in range(HO)])

            def halves(pt, lhsT, rhs, start, stop):
                for h in range(NH):
                    nc.tensor.matmul(
                        pt[:, h * MW:(h + 1) * MW], lhsT,
                        rhs[:, h * MW:(h + 1) * MW], start=start, stop=stop)

            # --- per-batch-tile pipeline ------------------------------------
            for t in range(NT):
                if t in pre_io:
                    mt = pre_io[t]
                else:
                    mt = spool.tile([8, BT], F16, name="mt", tag="mt", bufs=3)
                    nc.sync.dma_start(mt[:], m_d[:, t * BT:(t + 1) * BT])
                yt = mt[0:1, :]

                # ---- layer 0: single K=8 matmul per output block ----
                z = spool.tile([P, HO, BT], F16, name="z", tag="z", bufs=3)
                u = spool.tile([P, HO, BT], F16, name="u", tag="u", bufs=3)
                for mo in range(HO):
                    pz = psum.tile([P, BT], F32, name="pz", tag="ps")
                    halves(pz, w0[:, mo * P:(mo + 1) * P], mt[:], True, True)
                    nc.scalar.activation(z[:, mo, :], pz[:], RELU)
                for mo in range(HO):
                    pu = psum.tile([P, BT], F32, name="pu", tag="ps")
                    halves(pu, w0[:, H + mo * P:H + (mo + 1) * P],
                           mt[:], True, True)
                    nc.scalar.activation(u[:, mo, :], pu[:], RELU)

                # ---- layers 1..2 ----
                for i in (1, 2):
                    li = i - 1
                    # s = relu(u @ Wzuu[li] + bzuu[li]);  zz = z * s
                    zz = spool.tile([P, HO, BT], F16, name="zz", tag="zz",
                                    bufs=3)
                    srl = spool.tile([P, HO, BT], F16, name="srl", tag="srl",
                                     bufs=2)
                    for mo in range(HO):
                        ps = psum.tile([P, BT], F32, name="ps", tag="ps")
                        for ko in range(HO):
                            halves(ps, wb[:, li, ko, mo * P:(mo + 1) * P],
                                   u[:, ko, :], ko == 0, ko == HO - 1)
                        nc.scalar.activation(
                            srl[:, mo, :], ps[:], RELU,
                            bias=bzuu_t[:, li, mo:mo + 1])
                        nc.vector.tensor_mul(
                            zz[:, mo, :], z[:, mo, :], srl[:, mo, :])

                    # yy = y * (u @ Wyuu[li] + byuu[li]); the H-dim MAC runs
                    # on the DVE, only the cross-partition reduce uses the PE
                    tmp = py_dve(u, li)
                    q = psum.tile([P, BT], F32, name="q", tag="ps")
                    halves(q[0:1, :], ones128[:], tmp[:], True, True)
                    yy = spool.tile([1, BT], F16, name="yy", tag="yy", bufs=2)
                    nc.vector.scalar_tensor_tensor(
                        yy[:], q[0:1, :], byuu_t[:, li:li + 1], yt,
                        op0=ADD, op1=MULT)
                    # broadcast yy across all 128 partitions (gpsimd, which is
                    # otherwise idle) so the rank-1 yy@Wzyu[i] update can run
                    # on the DVE instead of costing a matmul pass per block
                    yyb = spool.tile([P, BT], F16, name="yyb", tag="yyb",
                                     bufs=2)
                    nc.gpsimd.partition_broadcast(yyb[:], yy[:])

                    # unew first: it is independent of zz/yy, keeps the PE
                    # busy while DVE produces zz and yy.
                    unew = spool.tile([P, HO, BT], F16, name="unew", tag="u",
                                      bufs=3)
                    for mo in range(HO):
                        pn = psum.tile([P, BT], F32, name="pn", tag="ps")
                        for ko in range(HO):
                            halves(pn, wb[:, 7 + li, ko, mo * P:(mo + 1) * P],
                                   u[:, ko, :], ko == 0, ko == HO - 1)
                        nc.scalar.activation(
                            unew[:, mo, :], pn[:], RELU,
                            bias=bu_t[:, li, mo:mo + 1])

                    znew = spool.tile([P, HO, BT], F16, name="znew", tag="z",
                                      bufs=3)
                    for mo in range(HO):
                        pt_ = psum.tile([P, BT], F32, name="pt", tag="ps")
                        for ko in range(HO):
                            halves(pt_, wb[:, 3 + li, ko, mo * P:(mo + 1) * P],
                                   zz[:, ko, :], ko == 0, False)
                        for ko in range(HO):
                            halves(pt_, wb[:, 5 + li, ko, mo * P:(mo + 1) * P],
                                   u[:, ko, :], False, ko == HO - 1)
                        zpre = spool.tile([P, BT], F32, name="zpre",
                                          tag="zpre", bufs=2)
                        nc.vector.scalar_tensor_tensor(
                            zpre[:], yyb[:], wzyv[:, li, mo:mo + 1], pt_[:],
                            op0=MULT, op1=ADD)
                        nc.scalar.activation(
                            znew[:, mo, :], zpre[:], RELU,
                            bias=bzu_t[:, li, mo:mo + 1])
                    z, u = znew, unew

                # ---- layer 3 (output) ----
                zz = spool.tile([P, HO, BT], F16, name="zz3", tag="zz", bufs=3)
                srl = spool.tile([P, HO, BT], F16, name="srl3", tag="srl",
                                 bufs=2)
                for mo in range(HO):
                    ps = psum.tile([P, BT], F32, name="ps3", tag="ps")
                    for ko in range(HO):
                        halves(ps, wb[:, 2, ko, mo * P:(mo + 1) * P],
                               u[:, ko, :], ko == 0, ko == HO - 1)
                    nc.scalar.activation(
                        srl[:, mo, :], ps[:], RELU, bias=bzuu_t[:, 2, mo:mo + 1])
                    nc.vector.tensor_mul(
                        zz[:, mo, :], z[:, mo, :], srl[:, mo, :])

                tmp3 = py_dve(u, 2)
                q3 = psum.tile([P, BT], F32, name="q3", tag="ps")
                halves(q3[0:1, :], ones128[:], tmp3[:], True, True)
                yy3 = spool.tile([1, BT], F32, name="yy3", tag="yy3", bufs=2)
                nc.vector.scalar_tensor_tensor(
                    yy3[:], q3[0:1, :], byuu_t[:, 2:3], yt,
                    op0=ADD, op1=MULT)

                tmpf = mac_chain(
                    [(zz[:, ho, :], wl3[:, ho, 0:1]) for ho in range(HO)]
                    + [(u[:, ho, :], wl3[:, ho, 1:2]) for ho in range(HO)])
                pf = psum.tile([P, BT], F32, name="pf", tag="ps")
                halves(pf[0:1, :], ones128[:], tmpf[:], True, True)
                # out = pf + yy3 + bzu_last  (Wzyu_last folded into wy/byuu)
                ot = spool.tile([1, BT], F32, name="ot", tag="ot", bufs=2)
                nc.vector.scalar_tensor_tensor(
                    ot[:], yy3[:], byuu_t[:, 3:4], pf[0:1, :],
                    op0=ADD, op1=ADD)
                nc.sync.dma_start(
                    out[t * BT:(t + 1) * BT, :].rearrange("b o -> o b"), ot[:])

    nc.compile()
    return nc


def _preprocess(inputs):
    """Fold scales into weights, build the replicated weight tensors."""
    f = lambda k: np.asarray(inputs[k], dtype=np.float32)
    p10 = 10.0 * np.float32(inputs["p"])
    pu10 = 10.0 * np.float32(inputs["pu"])
    pzu10 = 10.0 * np.float32(inputs["pzu"])

    wbig = np.concatenate([
        f("Wzuu") * pzu10,            # s=0..2
        f("Wzzu") * p10,              # s=3..4
        f("Wzu") * p10,               # s=5..6
        f("Wu") * pu10,               # s=7..8
    ], axis=0)

    # layer 0 stationary, K=8 rows [u(3), 1, y*u(3), y]
    Wzyu0 = f("Wzyu")[0]              # [1, H]
    w0z = np.concatenate([
        p10 * (f("byuu0")[None, :] @ Wzyu0),  # y row
        p10 * f("Wzu0"),                      # u rows
        p10 * f("bzu0")[None, :],             # ones row
        p10 * (f("Wyuu0") @ Wzyu0),           # y*u rows (rank-1 fold)
    ], axis=0)                                 # [8, H]
    w0u = np.concatenate([
        np.zeros((1, H), np.float32),
        pu10 * f("Wu0"),
        pu10 * f("bu0")[None, :],
        np.zeros((3, H), np.float32),
    ], axis=0)                                 # [8, H]
    w0 = np.concatenate([w0z, w0u], axis=1)    # [8, 2H]

    wy = f("Wyuu")[:, :, 0].transpose(1, 0).copy()       # [H, L]
    sc0 = f("Wzyu_last")[0, 0]
    wy[:, 2] *= sc0                                       # fold Wzyu_last
    wzyv = (f("Wzyu")[1:3, 0, :] * p10)                  # [2, H] f32
    wl3 = np.concatenate([f("Wzzu_last"), f("Wzu_last")], axis=1)  # [H, 2]

    bzuu = f("bzuu") * pzu10                   # [L, H]
    bu = f("bu") * pu10                        # [2, H]
    bzu = f("bzu") * p10                       # [2, H]
    byuu = np.array([[f("byuu")[0, 0], f("byuu")[1, 0],
                      sc0 * f("byuu")[2, 0], f("bzu_last")[0]]], np.float32)

    return dict(
        wbig=wbig.astype(np.float16), w0=w0.astype(np.float16),
        wy=wy.astype(np.float32), wzyv=wzyv.astype(np.float32),
        wl3=wl3.astype(np.float32),
        bzuu=bzuu, bu=bu, bzu=bzu, byuu=byuu,
    )


def _run(inputs, trace=False, **kw):
    if "nc" not in _CACHE:
        _CACHE["nc"] = _build()
    nc = _CACHE["nc"]
    shared = _preprocess(inputs)
    coords = np.asarray(inputs["coords"], dtype=np.float32)
    uT = coords[:, 0:D].T                       # [3, NFULL]
    yT = coords[:, D:D + 1].T                   # [1, NFULL]
    m = np.concatenate([
        yT, uT, np.ones((1, NFULL), np.float32), yT * uT,
    ], axis=0).astype(np.float16)               # [8, NFULL]; row 0 = y
    in_maps = [
        {**shared,
         "m": np.ascontiguousarray(m[:, c * NC_B:(c + 1) * NC_B])}
        for c in range(NCORES)
    ]
    res = run_bass_kernel_spmd(nc, in_maps, core_ids=list(range(NCORES)),
                               trace=trace, **kw)
    full = np.concatenate([res.results[c]["out"] for c in range(NCORES)], axis=0)
    return full, res


def kernel(**inputs) -> np.ndarray:
    out, _ = _run(inputs)
    return out


# revision 31
# speedup vs baseline: 1.1395x; 1.1395x over previous
"""Trainium2 Bass kernel for nn_FCBlock (dense MLP, 8-core data parallel).

Full (unsharded) inputs in, full output out. Internally: coords are sharded
along the batch axis across 8 NeuronCores, the (few-MB) weights are
replicated, and each core runs a fused 4-stage MLP entirely out of SBUF.

Math (per point, D=3, H=512, L=3):
  y = coords[:, 3:], u = coords[:, :3]
  z = relu(10p*(u@Wzu0 + bzu0 + (y*(u@Wyuu0+byuu0))@Wzyu[0]))
  u = relu(10pu*(u@Wu0 + bu0))
  for i in 1..3:
    zz = z * relu(10pzu*(u@Wzuu[i-1] + bzuu[i-1]))
    yy = y * (u@Wyuu[i-1] + byuu[i-1])
    t  = zz@Wzzu[i-1] + u@Wzu[i-1] + bzu[i-1] + yy@Wzyu[i]     (i<3)
    t  = zz@Wzzu_last + u@Wzu_last + bzu_last + yy@Wzyu_last   (i=3 -> output)
    z, u = relu(10p*t), relu(10pu*(u@Wu[i-1] + bu[i-1]))

Layer 0 is folded on the host into a single K=8 matmul: the moving tensor
carries rows [u, 1, y*u, y], so u@Wzu0 + bzu0 + (y*(u@Wyuu0+byuu0))@Wzyu[0]
and u@Wu0 + bu0 are each one matmul with all biases and the rank-1
Wyuu0@Wzyu0 outer product folded into the stationary.  The final
yy@Wzyu_last (1x1 weight) is folded into a DVE op.  All 10p/10pu/10pzu
scales are folded into weights/biases on the host.

Device layout: activations are [H (4x128 partitions), B_tile (free)], weights
stationary [K<=128, M<=128]. Matmuls emit in 512-wide moving slices (PSUM
bank limit); ACT/DVE epilogues run over the full BT-wide tile.
"""

import sys

sys.path.insert(0, "/opt/trn_rl_repo")

import numpy as np

import concourse.bacc as bacc
import concourse.mybir as mybir
from concourse.tile import TileContext
from concourse import bass_isa
from concourse.bass_utils import run_bass_kernel_spmd

NCORES = 8
NFULL = 65536
NC_B = NFULL // NCORES  # 8192 points per core
H = 512
L = 3
D = 3
P = 128
HO = H // P  # 4 partition-tiles per hidden vector
BT = 1024  # batch tile (ACT/DVE op width)
MW = 512   # matmul moving-slice width (PSUM bank limit)
NH = BT // MW
NT = NC_B // BT

F16 = mybir.dt.float16
F32 = mybir.dt.float32
RELU = mybir.ActivationFunctionType.Relu
ADD = mybir.AluOpType.add
MULT = mybir.AluOpType.mult
MAX = mybir.AluOpType.max

_CACHE = {}


def _build(fuse_srl=False):
    nc = bacc.Bacc(trn_type="TRN2", name="fcblock2")
    # fuse_srl: bzuu==0, so zz = max(psum,0)*z in one DVE op (no ACT pass)

    m_d = nc.dram_tensor("m", [8, NC_B], F16, kind="ExternalInput")
    wbig = nc.dram_tensor("wbig", [9, H, H], F16, kind="ExternalInput")
    w0_d = nc.dram_tensor("w0", [8, 2 * H], F16, kind="ExternalInput")
    wy_d = nc.dram_tensor("wy", [H, L], F32, kind="ExternalInput")
    wzyv_d = nc.dram_tensor("wzyv", [2, H], F32, kind="ExternalInput")
    wl3_d = nc.dram_tensor("wl3", [H, 2], F16, kind="ExternalInput")
    wlu_d = nc.dram_tensor("wlu", [H, 1], F32, kind="ExternalInput")
    bzuu_d = nc.dram_tensor("bzuu", [L, H], F32, kind="ExternalInput")
    bu_d = nc.dram_tensor("bu", [2, H], F32, kind="ExternalInput")
    bzu_d = nc.dram_tensor("bzu", [2, H], F32, kind="ExternalInput")
    byuu_d = nc.dram_tensor("byuu", [1, 4], F32, kind="ExternalInput")
    out = nc.dram_tensor("out", [NC_B, 1], F32, kind="ExternalOutput")

    with TileContext(nc) as tc:
        with (
            tc.tile_pool(name="wpool", bufs=1) as wpool,
            tc.tile_pool(name="spool", bufs=4) as spool,
            tc.tile_pool(name="psum", bufs=4, space="PSUM") as psum,
        ):
            # --- resident weights/biases ------------------------------------
            # Tile-0/1 inputs and w0 first: layer 0 of tile 0 needs only
            # these (tiny) DMAs, so they must not queue behind the rest.
            pre_io = {}
            for t in (0, 1):
                mt = spool.tile([8, BT], F16, name="mt", tag="mt", bufs=3)
                nc.sync.dma_start(mt[:], m_d[:, t * BT:(t + 1) * BT])
                pre_io[t] = mt
            w0 = wpool.tile([8, 2 * H], F16, name="w0")
            nc.sync.dma_start(w0[:], w0_d[:, :])
            wy = wpool.tile([P, HO, L], F32, name="wy")
            nc.sync.dma_start(wy[:], wy_d.rearrange("(ko p) s -> p ko s", p=P))
            wzyv = wpool.tile([P, 2, HO], F32, name="wzyv")
            nc.sync.dma_start(wzyv[:], wzyv_d.rearrange("s (ho p) -> p s ho", p=P))
            wl3 = wpool.tile([P, HO, 2], F16, name="wl3")
            nc.sync.dma_start(wl3[:], wl3_d.rearrange("(ko p) s -> p ko s", p=P))
            wlu = wpool.tile([P, HO, 1], F32, name="wlu")
            nc.sync.dma_start(wlu[:], wlu_d.rearrange("(ko p) s -> p ko s", p=P))
            bzuu_t = wpool.tile([P, L, HO], F32, name="bzuu_t")
            nc.sync.dma_start(bzuu_t[:], bzuu_d.rearrange("s (ho p) -> p s ho", p=P))
            bu_t = wpool.tile([P, 2, HO], F32, name="bu_t")
            nc.sync.dma_start(bu_t[:], bu_d.rearrange("s (ho p) -> p s ho", p=P))
            bzu_t = wpool.tile([P, 2, HO], F32, name="bzu_t")
            nc.sync.dma_start(bzu_t[:], bzu_d.rearrange("s (ho p) -> p s ho", p=P))
            byuu_t = wpool.tile([1, 4], F32, name="byuu_t")
            nc.sync.dma_start(byuu_t[:], byuu_d[:, :])
            ones128 = wpool.tile([P, 1], F16, name="ones128")
            nc.gpsimd.memset(ones128[:], 1.0)
            # Big dense weights, one DMA per layer-slice so the layer-1
            # consumers arrive before the tail ones.
            wb = wpool.tile([P, 9, HO, H], F16, name="wb")
            for s in (0, 3, 5, 7, 1, 4, 6, 8, 2):
                nc.sync.dma_start(
                    wb[:, s], wbig[s].rearrange("(ko p) m -> p ko m", p=P))

            def mac_chain(terms, tag="py"):
                # per-partition MAC chain on DVE: acc = sum_i tens_i*scal_i;
                # a single ones-column matmul then reduces across partitions.
                a = spool.tile([P, BT], F16, name=tag + "a", tag=tag + "a",
                               bufs=2)
                t0, s0 = terms[0]
                nc.vector.tensor_scalar_mul(a[:], t0, s0)
                for tn, sn in terms[1:]:
                    b = spool.tile([P, BT], F16, name=tag + "b", tag=tag + "b",
                                   bufs=2)
                    nc.vector.scalar_tensor_tensor(
                        b[:], tn, sn, a[:], op0=MULT, op1=ADD)
                    a = b
                return a

            def py_dve(u, col):
                return mac_chain(
                    [(u[:, ho, :], wy[:, ho, col:col + 1]) for ho in range(HO)])

            def halves(pt, lhsT, rhs, start, stop):
                for h in range(NH):
                    nc.tensor.matmul(
                        pt[:, h * MW:(h + 1) * MW], lhsT,
                        rhs[:, h * MW:(h + 1) * MW], start=start, stop=stop)

            # --- per-batch-tile pipeline ------------------------------------
            for t in range(NT):
                if t in pre_io:
                    mt = pre_io[t]
                else:
                    mt = spool.tile([8, BT], F16, name="mt", tag="mt", bufs=3)
                    nc.sync.dma_start(mt[:], m_d[:, t * BT:(t + 1) * BT])
                yt = mt[0:1, :]

                # ---- layer 0: single K=8 matmul per output block ----
                z = spool.tile([P, HO, BT], F16, name="z", tag="z", bufs=3)
                u = spool.tile([P, HO, BT], F16, name="u", tag="u", bufs=3)
                # L0 groups are only 2 matmuls each; alternate the relu
                # drains between ACT and DVE so the psum pool never gates the
                # PE (stalls here reset the p-state ramp to half clock).
                for mo in range(HO):
                    pz = psum.tile([P, BT], F32, name="pz", tag="ps")
                    halves(pz, w0[:, mo * P:(mo + 1) * P], mt[:], True, True)
                    if mo % 2 == 0:
                        nc.scalar.activation(z[:, mo, :], pz[:], RELU)
                    else:
                        nc.vector.tensor_scalar_max(z[:, mo, :], pz[:], 0.0)
                for mo in range(HO):
                    pu = psum.tile([P, BT], F32, name="pu", tag="ps")
                    halves(pu, w0[:, H + mo * P:H + (mo + 1) * P],
                           mt[:], True, True)
                    if mo % 2 == 0:
                        nc.scalar.activation(u[:, mo, :], pu[:], RELU)
                    else:
                        nc.vector.tensor_scalar_max(u[:, mo, :], pu[:], 0.0)

                # ---- layers 1..2 ----
                for i in (1, 2):
                    li = i - 1
                    # s = relu(u @ Wzuu[li] + bzuu[li]);  zz = z * s
                    zz = spool.tile([P, HO, BT], F16, name="zz", tag="zz",
                                    bufs=3)
                    srl = spool.tile([P, HO, BT], F16, name="srl", tag="srl",
                                     bufs=2)
                    for mo in range(HO):
                        ps = psum.tile([P, BT], F32, name="ps", tag="ps")
                        for ko in range(HO):
                            halves(ps, wb[:, li, ko, mo * P:(mo + 1) * P],
                                   u[:, ko, :], ko == 0, ko == HO - 1)
                        if fuse_srl:
                            nc.vector.scalar_tensor_tensor(
                                zz[:, mo, :], ps[:], 0.0, z[:, mo, :],
                                op0=MAX, op1=MULT)
                        else:
                            nc.scalar.activation(
                                srl[:, mo, :], ps[:], RELU,
                                bias=bzuu_t[:, li, mo:mo + 1])
                            nc.vector.tensor_mul(
                                zz[:, mo, :], z[:, mo, :], srl[:, mo, :])

                    # yy = y * (u @ Wyuu[li] + byuu[li]); the H-dim MAC runs
                    # on the DVE, only the cross-partition reduce uses the PE
                    tmp = py_dve(u, li)
                    q = psum.tile([P, BT], F32, name="q", tag="ps")
                    halves(q[0:1, :], ones128[:], tmp[:], True, True)
                    yy = spool.tile([1, BT], F16, name="yy", tag="yy", bufs=2)
                    nc.vector.scalar_tensor_tensor(
                        yy[:], q[0:1, :], byuu_t[:, li:li + 1], yt,
                        op0=ADD, op1=MULT)
                    # broadcast yy across all 128 partitions (gpsimd, which is
                    # otherwise idle) so the rank-1 yy@Wzyu[i] update can run
                    # on the DVE instead of costing a matmul pass per block
                    yyb = spool.tile([P, BT], F16, name="yyb", tag="yyb",
                                     bufs=2)
                    nc.gpsimd.partition_broadcast(yyb[:], yy[:])

                    # unew first: it is independent of zz/yy, keeps the PE
                    # busy while DVE produces zz and yy.
                    unew = spool.tile([P, HO, BT], F16, name="unew", tag="u",
                                      bufs=3)
                    for mo in range(HO):
                        pn = psum.tile([P, BT], F32, name="pn", tag="ps")
                        for ko in range(HO):
                            halves(pn, wb[:, 7 + li, ko, mo * P:(mo + 1) * P],
                                   u[:, ko, :], ko == 0, ko == HO - 1)
                        nc.scalar.activation(
                            unew[:, mo, :], pn[:], RELU,
                            bias=bu_t[:, li, mo:mo + 1])

                    znew = spool.tile([P, HO, BT], F16, name="znew", tag="z",
                                      bufs=3)
                    for mo in range(HO):
                        pt_ = psum.tile([P, BT], F32, name="pt", tag="ps")
                        for ko in range(HO):
                            halves(pt_, wb[:, 3 + li, ko, mo * P:(mo + 1) * P],
                                   zz[:, ko, :], ko == 0, False)
                        for ko in range(HO):
                            halves(pt_, wb[:, 5 + li, ko, mo * P:(mo + 1) * P],
                                   u[:, ko, :], False, ko == HO - 1)
                        zpre = spool.tile([P, BT], F32, name="zpre",
                                          tag="zpre", bufs=2)
                        nc.vector.scalar_tensor_tensor(
                            zpre[:], yyb[:], wzyv[:, li, mo:mo + 1], pt_[:],
                            op0=MULT, op1=ADD)
                        nc.scalar.activation(
                            znew[:, mo, :], zpre[:], RELU,
                            bias=bzu_t[:, li, mo:mo + 1])
                    z, u = znew, unew

                # ---- layer 3 (output) ----
                zz = spool.tile([P, HO, BT], F16, name="zz3", tag="zz", bufs=3)
                srl = spool.tile([P, HO, BT], F16, name="srl3", tag="srl",
                                 bufs=2)
                for mo in range(HO):
                    ps = psum.tile([P, BT], F32, name="ps3", tag="ps")
                    for ko in range(HO):
                        halves(ps, wb[:, 2, ko, mo * P:(mo + 1) * P],
                               u[:, ko, :], ko == 0, ko == HO - 1)
                    if fuse_srl:
                        nc.vector.scalar_tensor_tensor(
                            zz[:, mo, :], ps[:], 0.0, z[:, mo, :],
                            op0=MAX, op1=MULT)
                    else:
                        nc.scalar.activation(
                            srl[:, mo, :], ps[:], RELU,
                            bias=bzuu_t[:, 2, mo:mo + 1])
                        nc.vector.tensor_mul(
                            zz[:, mo, :], z[:, mo, :], srl[:, mo, :])

                tmp3 = py_dve(u, 2)
                q3 = psum.tile([P, BT], F32, name="q3", tag="ps")
                halves(q3[0:1, :], ones128[:], tmp3[:], True, True)
                yy3 = spool.tile([1, BT], F32, name="yy3", tag="yy3", bufs=2)
                nc.vector.scalar_tensor_tensor(
                    yy3[:], q3[0:1, :], byuu_t[:, 2:3], yt,
                    op0=ADD, op1=MULT)

                pf = psum.tile([P, BT], F32, name="pf", tag="ps")
                halves(pf[0:1, :], ones128[:], tmpu[:], True, False)
                for ko in range(HO):
                    halves(pf[0:1, :], wl3[:, ko, 0:1], zz[:, ko, :],
                           False, ko == HO - 1)
                # out = pf + yy3 + bzu_last  (Wzyu_last folded into wy/byuu)
                ot = spool.tile([1, BT], F32, name="ot", tag="ot", bufs=2)
                nc.vector.scalar_tensor_tensor(
                    ot[:], yy3[:], byuu_t[:, 3:4], pf[0:1, :],
                    op0=ADD, op1=ADD)
                nc.sync.dma_start(
                    out[t * BT:(t + 1) * BT, :].rearrange("b o -> o b"), ot[:])

    nc.compile()
    return nc


def _preprocess(inputs):
    """Fold scales into weights, build the replicated weight tensors."""
    f = lambda k: np.asarray(inputs[k], dtype=np.float32)
    p10 = 10.0 * np.float32(inputs["p"])
    pu10 = 10.0 * np.float32(inputs["pu"])
    pzu10 = 10.0 * np.float32(inputs["pzu"])

    wbig = np.concatenate([
        f("Wzuu") * pzu10,            # s=0..2
        f("Wzzu") * p10,              # s=3..4
        f("Wzu") * p10,               # s=5..6
        f("Wu") * pu10,               # s=7..8
    ], axis=0)

    # layer 0 stationary, K=8 rows [u(3), 1, y*u(3), y]
    Wzyu0 = f("Wzyu")[0]              # [1, H]
    w0z = np.concatenate([
        p10 * (f("byuu0")[None, :] @ Wzyu0),  # y row
        p10 * f("Wzu0"),                      # u rows
        p10 * f("bzu0")[None, :],             # ones row
        p10 * (f("Wyuu0") @ Wzyu0),           # y*u rows (rank-1 fold)
    ], axis=0)                                 # [8, H]
    w0u = np.concatenate([
        np.zeros((1, H), np.float32),
        pu10 * f("Wu0"),
        pu10 * f("bu0")[None, :],
        np.zeros((3, H), np.float32),
    ], axis=0)                                 # [8, H]
    w0 = np.concatenate([w0z, w0u], axis=1)    # [8, 2H]

    wy = f("Wyuu")[:, :, 0].transpose(1, 0).copy()       # [H, L]
    sc0 = f("Wzyu_last")[0, 0]
    wy[:, 2] *= sc0                                       # fold Wzyu_last
    wzyv = (f("Wzyu")[1:3, 0, :] * p10)                  # [2, H] f32
    wl3 = np.concatenate([f("Wzzu_last"), f("Wzu_last")], axis=1)  # [H, 2]

    bzuu = f("bzuu") * pzu10                   # [L, H]
    bu = f("bu") * pu10                        # [2, H]
    bzu = f("bzu") * p10                       # [2, H]
    byuu = np.array([[f("byuu")[0, 0], f("byuu")[1, 0],
                      sc0 * f("byuu")[2, 0], f("bzu_last")[0]]], np.float32)

    return dict(
        wbig=wbig.astype(np.float16), w0=w0.astype(np.float16),
        wy=wy.astype(np.float32), wzyv=wzyv.astype(np.float32),
        wl3=wl3.astype(np.float16), wlu=f("Wzu_last").astype(np.float32),
        bzuu=bzuu, bu=bu, bzu=bzu, byuu=byuu,
    )


def _run(inputs, trace=False, **kw):
    # srl fusion measured slower (DVE is in the psum drain path already)
    fuse_srl = False
    key = ("nc", fuse_srl)
    if key not in _CACHE:
        _CACHE[key] = _build(fuse_srl)
    nc = _CACHE[key]
    shared = _preprocess(inputs)
    coords = np.asarray(inputs["coords"], dtype=np.float32)
    uT = coords[:, 0:D].T                       # [3, NFULL]
    yT = coords[:, D:D + 1].T                   # [1, NFULL]
    m = np.concatenate([
        yT, uT, np.ones((1, NFULL), np.float32), yT * uT,
    ], axis=0).astype(np.float16)               # [8, NFULL]; row 0 = y
    in_maps = [
        {**shared,
         "m": np.ascontiguousarray(m[:, c * NC_B:(c + 1) * NC_B])}
        for c in range(NCORES)
    ]
    res = run_bass_kernel_spmd(nc, in_maps, core_ids=list(range(NCORES)),
                               trace=trace, **kw)
    full = np.concatenate([res.results[c]["out"] for c in range(NCORES)], axis=0)
    return full, res


def kernel(**inputs) -> np.ndarray:
    out, _ = _run(inputs)
    return out
